# revision 7
# baseline (speedup 1.0000x reference)
"""Distributed memory-shard scale kernel for Trainium2 (8 NeuronCores).

Computes out[b, s, d] = x[b, s, d] * shards[shard_map[d], d] for
x: [4, 4096, 4096] f32, shards: [8, 4096] f32, shard_map: [4096] int.

Strategy: data-parallel over the flattened (batch*seq) rows — each of the
8 cores owns a contiguous 2048-row slice of x. The per-dim weight vector
w[d] = shards[shard_map[d], d] is gathered on the host (it is 16 KB; the
256 MB x-scaling stays on device) and passed to every core, so the device
preamble is just: load w row -> replicate to 128 partitions with K=1
outer-product matmuls (PE + DVE only).

The stream phase is fabric-bound (~436 GB/s SBUF AXI ceiling per core,
loads + stores combined). Perfetto analysis of the previous version
showed two stalls: the store queue idle for ~15 us waiting on a slow
on-device w build, and a ~48 us store-only tail (store-only traffic
runs at ~300-360 GB/s vs 433 GB/s mixed) because loads ran 5 big tiles
ahead and finished early. This version starts stores by ~12 us and
keeps the load lead small (bufs=4 x 2MB) so both DMA queues stay fed
until the end: 16 tiles of [128 rows, 4096 dims] (2 MB) loaded on the
sync HWDGE ring, multiplied by w in halves on DVE, and stored as 1 MB
halves on the scalar HWDGE ring (last tile in 512 KB quarters to
shorten the drain).
"""

import numpy as np

import bass_rust as _bass_rust
import concourse.bass as bass
import concourse.tile as tile
from concourse import mybir
from concourse.bass_utils import run_bass_kernel_spmd

N_CORES = 8
BATCH, SEQ, DIM = 4, 4096, 4096
NUM_SHARDS = 8
ROWS_TOTAL = BATCH * SEQ               # 16384
ROWS_PER_CORE = ROWS_TOTAL // N_CORES  # 2048
P = 128                                # SBUF partitions
N_TILES = ROWS_PER_CORE // P           # 16 tiles of [128, 4096]
HW = DIM // 2                          # half width (2048)
QW = DIM // 4                          # quarter width (1024)
BUFS = 4

TRACE = False       # set True (e.g. from test.py) to capture an NTFF profile
LAST_RESULT = None  # BassKernelResults of the most recent kernel() call

_cached_nc = None


def _build_program() -> bass.Bass:
    f32 = mybir.dt.float32
    nc = bass.Bass()
    x_in = nc.dram_tensor("x", [ROWS_PER_CORE, DIM], f32, kind="ExternalInput")
    w_in = nc.dram_tensor("w", [1, DIM], f32, kind="ExternalInput")
    out = nc.dram_tensor("out", [ROWS_PER_CORE, DIM], f32,
                         kind="ExternalOutput")

    with tile.TileContext(nc) as tc:
        with tc.tile_pool(name="const", bufs=1) as cpool, \
             tc.tile_pool(name="xp", bufs=BUFS) as xpool:
            # ones row for the broadcast matmuls — engine op, no DMA
            ones = cpool.tile([1, P], f32)
            nc.vector.memset(ones[:], 1.0)
            # w row load FIRST on the sync ring: the scalar sequencer's
            # startup runs ~3 us behind sync, and w gates the first
            # mul+store — the 16 KB it costs the x stream is noise.
            w128 = cpool.tile([P, DIM], f32)
            wrow = w128[0:1, :]
            nc.sync.dma_start(wrow, w_in[:])
            # replicate w to all 128 partitions: ones[1,128].T @
            # wrow[1,512] -> PSUM[128,512], copy back on DVE.
            MMF = 512  # one PSUM bank per matmul
            with tc.tile_pool(name="ps", bufs=8, space="PSUM") as ppool:
                for k in range(DIM // MMF):
                    mm = ppool.tile([P, MMF], f32)
                    nc.tensor.matmul(mm[:], ones[:],
                                     w128[0:1, k * MMF:(k + 1) * MMF],
                                     start=True, stop=True)
                    nc.vector.tensor_copy(w128[:, k * MMF:(k + 1) * MMF],
                                          mm[:])

            # --- stream x through SBUF, scaling by w ---
            x3v = x_in.rearrange("(i p) d -> i p d", p=P)
            o3v = out.rearrange("(i p) d -> i p d", p=P)
            for i in range(N_TILES):
                xt = xpool.tile([P, DIM], f32)
                if i == 0 or i == N_TILES - 1:
                    # first tile: quarter-column loads+muls+stores so the
                    # first store only waits on a 512 KB load and
                    # w128[:, :1024] (2 matmul+copies); last tile:
                    # quarters keep the final load->mul->store chain
                    # short
                    for q in range(4):
                        cols = slice(q * QW, (q + 1) * QW)
                        nc.sync.dma_start(xt[:, cols], x3v[i, :, cols])
                        nc.vector.tensor_mul(xt[:, cols], xt[:, cols],
                                             w128[:, cols])
                        nc.scalar.dma_start(o3v[i, :, cols], xt[:, cols])
                else:
                    nc.sync.dma_start(xt[:], x3v[i])
                    for h in range(2):
                        cols = slice(h * HW, (h + 1) * HW)
                        nc.vector.tensor_mul(xt[:, cols], xt[:, cols],
                                             w128[:, cols])
                        nc.scalar.dma_start(o3v[i, :, cols], xt[:, cols])
    # TRN2 allows one sync wait per instruction; split multi-wait
    # instructions the way bacc's compile pipeline does.
    _bass_rust.generate_event_semaphores(nc)
    return nc


def kernel(x, shards, shard_map):
    global _cached_nc, LAST_RESULT
    if _cached_nc is None:
        _cached_nc = _build_program()
    nc = _cached_nc

    x2 = np.asarray(x, dtype=np.float32).reshape(ROWS_TOTAL, DIM)
    sh = np.asarray(shards, dtype=np.float32)
    sm = np.asarray(shard_map).astype(np.int64)
    w = sh[sm, np.arange(DIM)].reshape(1, DIM).astype(np.float32)

    in_maps = [
        {"x": x2[c * ROWS_PER_CORE:(c + 1) * ROWS_PER_CORE], "w": w}
        for c in range(N_CORES)
    ]
    res = run_bass_kernel_spmd(nc, in_maps, core_ids=list(range(N_CORES)),
                               trace=TRACE)
    LAST_RESULT = res
    return np.concatenate([r["out"] for r in res.results],
                          axis=0).reshape(BATCH, SEQ, DIM)
